# revision 7
# baseline (speedup 1.0000x reference)
"""DLRM forward on 8 Trainium2 NeuronCores (Bass/Tile).

Strategy (v5):
- True-z-order sharding: core m owns z rows [54m, 54m+54) (z = dense(16)
  | 26 tables x 16). Each core gathers its <=4 tables (4 indirect DMAs
  per table, one per 128-batch block), PE-transposes to [16, 512], and
  scatters the rows it owns into ag_in via one indirect DMA. AllGather
  then yields ag_out = full z in TRUE row order, so the interaction
  j-sources are plain strided DMAs (no post-AG gathers except ziT).
- Interaction + pred layer 0 use host-folded symmetric weights in fp8:
  out[o,b] = sum_{i<=j} Wf[(i,j),o] z_i[b] z_j[b]. Core c owns i-rows
  {128q+8d+c} u {384+8d+c}. 100 exact-packed 128-row chunks (7% pad),
  50 DoubleRow K=256 pairs, 4 PSUM banks. DVE builds bf16 products
  (mostly vector engine, 2x mode); scalar/gpsimd convert to fp8 in
  2-pair batches; PE runs back-to-back DoubleRow matmuls (HAM-warm).
- Partial out^T is AllReduced (bf16); every core computes the pred tail
  (relu on vector/gpsimd, sigmoid table preloaded) redundantly.
"""

import numpy as np
import ml_dtypes

BATCH = 512
CARD = 100000
ED = 16
NCORES = 8
NIL = 54
ZR = 432
O = 512

BF16 = ml_dtypes.bfloat16
F8 = ml_dtypes.float8_e4m3fn

_state = {}


def _jstart(il):
    if il < 48:
        q, dd = divmod(il, 16)
        return 128 * q + 8 * dd
    return 384 + 8 * (il - 48)


def _i_of(il, c):
    return _jstart(il) + c


def _src_j(src, p):
    if src.startswith("zt"):
        return 128 * int(src[2]) + p
    if src == "t3a":
        return 384 + (p % 48)
    return 384 + ((p + 24) % 48)  # t3b


def _build_recipe():
    """100 chunks (50 pairs): see recipe analysis.

    Returns (chunks, jmap, ilmap): chunks = list of segment lists
    (p0, plen, src, il); jmap/ilmap [100,128], jmap=-1 where unused.
    """
    pieces = [(il, 0, 48) for il in range(49)]
    pieces += [(48 + d, 8 * d, 48 - 8 * d) for d in range(1, 6)]

    diag_windows = []
    for dd in range(16):
        for q in range(3):
            for k in range(8 * dd // 48):
                diag_windows.append((q, dd, 48 * k))
    diag_windows.sort(key=lambda w: (w[1], w[0], w[2]))
    whole = [p for p in pieces if p[2] == 48]
    rest = [p for p in pieces if p[2] < 48]
    gap_fill = {}
    for (q, dd, p0), (il, lo, ln) in zip(diag_windows, whole[:42]):
        gap_fill.setdefault((q, dd), []).append((p0, 48, "t3a", il))
    leftover = whole[42:] + rest

    tail_chunks = [[] for _ in range(4)]
    slots = []
    for ci in range(4):
        slots += [(ci, 0), (ci, 48)]
    part_ps = [p for p in leftover if p[1] > 0]
    full_ps = [p for p in leftover if p[1] == 0]
    fi = 0
    for (ci, w0) in slots[:7]:
        il, lo, ln = full_ps[fi]
        tail_chunks[ci].append((w0, 48, "t3a", il))
        fi += 1
    il, lo, ln = part_ps[0]
    tail_chunks[3].append((48 + lo, ln, "t3a", il))

    part_regions = []
    for dd in range(16):
        L = 8 * dd % 48
        if L:
            for q in range(3):
                part_regions.append((q, dd, 48 * (8 * dd // 48), L))
    part_regions.sort(key=lambda r: -r[3])
    atoms = {}
    for (il2, lo2, ln2) in part_ps[1:]:
        atoms[il2] = list(range(lo2, 48))
    for (q, dd, p0, L) in part_regions:
        if not atoms:
            break
        cands = [a for a, rs in atoms.items() if rs and rs[0] < L]
        if not cands:
            continue
        a = min(cands, key=lambda x: atoms[x][0])
        rs = atoms[a]
        take = [r for r in rs if r < L]
        if not take:
            continue
        lo3, hi3 = take[0], take[-1]
        gap_fill.setdefault((q, dd), []).append(
            (p0 + lo3, hi3 + 1 - lo3, "t3a", a))
        atoms[a] = [r for r in rs if r >= L]
        if not atoms[a]:
            del atoms[a]
    for ci, a in enumerate(sorted(atoms)):
        tail_chunks[ci].append((112, 8, "t3b", a))

    chunks = []
    for il in range(16):
        chunks.append([(0, 128, "zt1", il)])
        chunks.append([(0, 128, "zt2", il)])
    for il in range(16, 32):
        chunks.append([(0, 128, "zt2", il)])
    for dd in range(16):
        for q in range(3):
            il = 16 * q + dd
            ops = [(8 * dd, 128 - 8 * dd, f"zt{q}", il)]
            ops += gap_fill.get((q, dd), [])
            ops.sort()
            chunks.append(ops)
    for ops in tail_chunks:
        ops.sort()
        chunks.append(ops)

    nch = len(chunks)
    jmap = -np.ones((nch, 128), np.int64)
    ilmap = np.zeros((nch, 128), np.int64)
    for t, ops in enumerate(chunks):
        for (p0, plen, src, il4) in ops:
            for e in range(plen):
                p = p0 + e
                jmap[t, p] = _src_j(src, p)
                ilmap[t, p] = il4
    return chunks, jmap, ilmap


CHUNKS, JMAP, ILMAP = _build_recipe()
NCH = len(CHUNKS)
NPAIR = NCH // 2
NDBL = NPAIR // 2   # double-tiles of 4 chunks
NBUF = 4            # ring of double-tiles


def _build_module():
    import concourse.bass as bass
    import concourse.mybir as mybir
    import concourse.tile as tile
    from concourse import bacc
    from concourse.masks import make_identity

    dt = mybir.dt
    nc = bacc.Bacc("TRN2", target_bir_lowering=False, debug=False,
                   num_devices=NCORES)

    wsb_d = nc.dram_tensor("wsb", [128, 2, NPAIR * O], dt.float8e4,
                           kind="ExternalInput").ap()
    zidx = nc.dram_tensor("zidx", [NIL, 1], dt.int32, kind="ExternalInput").ap()
    pidx = nc.dram_tensor("pidx", [96, 1], dt.int32, kind="ExternalInput").ap()
    embs = {}
    for j in range(5):
        embs[j] = nc.dram_tensor(f"es{j}", [CARD, ED], dt.bfloat16,
                                 kind="ExternalInput").ap()
    idxq = nc.dram_tensor("idxq", [128, 20], dt.int32, kind="ExternalInput").ap()
    dfT = nc.dram_tensor("dfT", [16, BATCH], dt.bfloat16, kind="ExternalInput").ap()
    dw0 = nc.dram_tensor("dw0", [16, 512], dt.bfloat16, kind="ExternalInput").ap()
    dw1 = nc.dram_tensor("dw1", [512, 256], dt.bfloat16, kind="ExternalInput").ap()
    dw2 = nc.dram_tensor("dw2", [256, 64], dt.bfloat16, kind="ExternalInput").ap()
    dw3 = nc.dram_tensor("dw3", [64, 16], dt.bfloat16, kind="ExternalInput").ap()
    db0q = nc.dram_tensor("db0q", [128, 4], dt.float32, kind="ExternalInput").ap()
    db1q = nc.dram_tensor("db1q", [128, 2], dt.float32, kind="ExternalInput").ap()
    db2q = nc.dram_tensor("db2q", [64, 1], dt.float32, kind="ExternalInput").ap()
    db3q = nc.dram_tensor("db3q", [16, 1], dt.float32, kind="ExternalInput").ap()
    pw1 = nc.dram_tensor("pw1", [512, 256], dt.bfloat16, kind="ExternalInput").ap()
    pw2 = nc.dram_tensor("pw2", [256, 1], dt.bfloat16, kind="ExternalInput").ap()
    pb0q = nc.dram_tensor("pb0q", [128, 4], dt.float32, kind="ExternalInput").ap()
    pb1q = nc.dram_tensor("pb1q", [128, 2], dt.float32, kind="ExternalInput").ap()
    pb2q = nc.dram_tensor("pb2q", [1, 1], dt.float32, kind="ExternalInput").ap()
    out_d = nc.dram_tensor("out", [1, BATCH], dt.float32, kind="ExternalOutput").ap()

    rg = [list(range(NCORES))]

    with tile.TileContext(nc) as tc:
        with tc.tile_pool(name="const", bufs=1) as cp, \
             tc.tile_pool(name="ip", bufs=6) as ip, \
             tc.tile_pool(name="dram", bufs=1, space="DRAM") as dp:

            ag_in = dp.tile([NIL, BATCH], dt.bfloat16, tag="ag_in", name="ag_in")
            ag_out = dp.tile([ZR, BATCH], dt.bfloat16, tag="ag_out", name="ag_out")
            zi_d = dp.tile([NIL, BATCH], dt.bfloat16, tag="zi_d", name="zi_d")
            ar_in = dp.tile([O, BATCH], dt.bfloat16, tag="ar_in", name="ar_in")
            ar_out = dp.tile([O, BATCH], dt.bfloat16, tag="ar_out", name="ar_out")

            wsb = cp.tile([128, 2, NPAIR * O], dt.float8e4, tag="wsb", name="wsb")

            ident = cp.tile([128, 128], dt.bfloat16, tag="ident", name="ident")
            make_identity(nc, ident[:])
            idx_sb = cp.tile([128, 20], dt.int32, tag="idx_sb", name="idx_sb")
            nc.sync.dma_start(out=idx_sb[:], in_=idxq[:, :])
            zidx_sb = cp.tile([NIL, 1], dt.int32, tag="zidx_sb", name="zidx_sb")
            nc.sync.dma_start(out=zidx_sb[:], in_=zidx[:, :])
            pidx_sb = cp.tile([96, 1], dt.int32, tag="pidx_sb", name="pidx_sb")
            nc.sync.dma_start(out=pidx_sb[:], in_=pidx[:, :])
            dfT_sb = cp.tile([16, BATCH], dt.bfloat16, tag="dfT_sb", name="dfT_sb")
            nc.sync.dma_start(out=dfT_sb[:], in_=dfT[:, :])
            dw0_sb = cp.tile([16, 512], dt.bfloat16, tag="dw0_sb", name="dw0_sb")
            nc.sync.dma_start(out=dw0_sb[:], in_=dw0[:, :])
            dw1_sb = [cp.tile([128, 256], dt.bfloat16, tag=f"dw1_{k}", name=f"dw1_{k}")
                      for k in range(4)]
            for k in range(4):
                nc.sync.dma_start(out=dw1_sb[k][:], in_=dw1[k * 128:(k + 1) * 128, :])
            dw2_sb = [cp.tile([128, 64], dt.bfloat16, tag=f"dw2_{k}", name=f"dw2_{k}")
                      for k in range(2)]
            for k in range(2):
                nc.sync.dma_start(out=dw2_sb[k][:], in_=dw2[k * 128:(k + 1) * 128, :])
            dw3_sb = cp.tile([64, 16], dt.bfloat16, tag="dw3_sb", name="dw3_sb")
            nc.sync.dma_start(out=dw3_sb[:], in_=dw3[:, :])
            pw1_sb = [cp.tile([128, 256], dt.bfloat16, tag=f"pw1_{k}", name=f"pw1_{k}")
                      for k in range(4)]
            for k in range(4):
                nc.sync.dma_start(out=pw1_sb[k][:], in_=pw1[k * 128:(k + 1) * 128, :])
            pw2_sb = [cp.tile([128, 1], dt.bfloat16, tag=f"pw2_{k}", name=f"pw2_{k}")
                      for k in range(2)]
            for k in range(2):
                nc.sync.dma_start(out=pw2_sb[k][:], in_=pw2[k * 128:(k + 1) * 128, :])
            db0_sb = cp.tile([128, 4], dt.float32, tag="db0_sb", name="db0_sb")
            nc.sync.dma_start(out=db0_sb[:], in_=db0q[:, :])
            db1_sb = cp.tile([128, 2], dt.float32, tag="db1_sb", name="db1_sb")
            nc.sync.dma_start(out=db1_sb[:], in_=db1q[:, :])
            db2_sb = cp.tile([64, 1], dt.float32, tag="db2_sb", name="db2_sb")
            nc.sync.dma_start(out=db2_sb[:], in_=db2q[:, :])
            db3_sb = cp.tile([16, 1], dt.float32, tag="db3_sb", name="db3_sb")
            nc.sync.dma_start(out=db3_sb[:], in_=db3q[:, :])
            pb0_sb = cp.tile([128, 4], dt.float32, tag="pb0_sb", name="pb0_sb")
            nc.sync.dma_start(out=pb0_sb[:], in_=pb0q[:, :])
            pb1_sb = cp.tile([128, 2], dt.float32, tag="pb1_sb", name="pb1_sb")
            nc.sync.dma_start(out=pb1_sb[:], in_=pb1q[:, :])
            pb2_sb = cp.tile([1, 1], dt.float32, tag="pb2_sb", name="pb2_sb")
            nc.sync.dma_start(out=pb2_sb[:], in_=pb2q[:, :])

            # weight slab prefetch on the scalar HWDGE ring
            PFC = 8
            for t0 in range(0, NPAIR, PFC):
                t1 = min(NPAIR, t0 + PFC)
                nc.scalar.dma_start(out=wsb[:, :, t0 * O:t1 * O],
                                    in_=wsb_d[:, :, t0 * O:t1 * O])

            # itb double-tile ring; pre-zero once (gap rows stay finite;
            # fp8 'it' gap rows hit zero weights so stale data is fine)
            itbs = [ip.tile([128, 4, BATCH], dt.bfloat16, tag="itb",
                            name="itb") for _ in range(NBUF)]
            for t_ in itbs:
                nc.vector.memset(t_[:, :, :], 0)

            # ---- dense MLP + gathers + scatter ----
            with tc.tile_pool(name="gather", bufs=2) as gp, \
                 tc.tile_pool(name="ps_g", bufs=2, space="PSUM") as pg, \
                 tc.tile_pool(name="ps_d", bufs=4, space="PSUM") as pd, \
                 tc.tile_pool(name="dmlp", bufs=1) as dm:
                h1 = []
                for mc in range(4):
                    d1 = pd.tile([128, BATCH], dt.float32, tag="dscr", name="d1")
                    nc.tensor.matmul(out=d1[:], lhsT=dw0_sb[:, mc * 128:(mc + 1) * 128],
                                     rhs=dfT_sb[:], start=True, stop=True)
                    h = dm.tile([128, BATCH], dt.bfloat16, tag=f"h1_{mc}",
                                name=f"h1_{mc}")
                    nc.scalar.activation(out=h[:], in_=d1[:],
                                         func=mybir.ActivationFunctionType.Relu,
                                         bias=db0_sb[:, mc:mc + 1])
                    h1.append(h)
                h2 = []
                for mc in range(2):
                    d2 = pd.tile([128, BATCH], dt.float32, tag="dscr", name="d2")
                    for k in range(4):
                        nc.tensor.matmul(out=d2[:],
                                         lhsT=dw1_sb[k][:, mc * 128:(mc + 1) * 128],
                                         rhs=h1[k][:], start=(k == 0), stop=(k == 3))
                    h = dm.tile([128, BATCH], dt.bfloat16, tag=f"h2_{mc}",
                                name=f"h2_{mc}")
                    nc.scalar.activation(out=h[:], in_=d2[:],
                                         func=mybir.ActivationFunctionType.Relu,
                                         bias=db1_sb[:, mc:mc + 1])
                    h2.append(h)
                d3 = pd.tile([128, BATCH], dt.float32, tag="dscr", name="d3")
                for k in range(2):
                    nc.tensor.matmul(out=d3[:64, :], lhsT=dw2_sb[k][:, :],
                                     rhs=h2[k][:], start=(k == 0), stop=(k == 1))
                h3 = dm.tile([64, BATCH], dt.bfloat16, tag="h3", name="h3")
                nc.scalar.activation(out=h3[:], in_=d3[:64, :],
                                     func=mybir.ActivationFunctionType.Relu,
                                     bias=db2_sb[:, 0:1])
                d4 = pd.tile([128, BATCH], dt.float32, tag="dscr", name="d4")
                nc.tensor.matmul(out=d4[:16, :], lhsT=dw3_sb[:, :], rhs=h3[:],
                                 start=True, stop=True)

                # z_all: rows 0-15 dense, 16+16j table slot j
                z_all = cp.tile([96, BATCH], dt.bfloat16, tag="z_all",
                                name="z_all")
                nc.vector.tensor_scalar_add(out=z_all[0:16, :], in0=d4[:16, :],
                                            scalar1=db3_sb[:, 0:1])

                # preload sigmoid ACT table while scalar is idle (dense
                # relus done; loop converts use Copy which needs no set)
                sig_scr = dm.tile([1, 1], dt.float32, tag="sig_scr",
                                  name="sig_scr")
                nc.scalar.activation(out=sig_scr[:], in_=pb2_sb[:, :],
                                     func=mybir.ActivationFunctionType.Sigmoid)

                for j in range(5):
                    ps_s = pg.tile([16, BATCH], dt.bfloat16, tag="psg",
                                   name=f"psg{j}")
                    for bc in range(4):
                        gt = gp.tile([128, ED], dt.bfloat16, tag=f"g{j}_{bc}",
                                     name=f"g{j}_{bc}")
                        nc.gpsimd.indirect_dma_start(
                            out=gt[:], out_offset=None, in_=embs[j][:, :],
                            in_offset=bass.IndirectOffsetOnAxis(
                                ap=idx_sb[:, j * 4 + bc:j * 4 + bc + 1], axis=0))
                        nc.tensor.transpose(out=ps_s[:16, bc * 128:(bc + 1) * 128],
                                            in_=gt[:, :], identity=ident[:])
                    z_s = dm.tile([16, BATCH], dt.bfloat16, tag=f"zsl{j}",
                                  name=f"zsl{j}")
                    if j % 2 == 0:
                        nc.vector.tensor_copy(out=z_s[:], in_=ps_s[:16, :])
                    else:
                        nc.scalar.activation(
                            out=z_s[:], in_=ps_s[:16, :],
                            func=mybir.ActivationFunctionType.Copy)
                    # partition-shifting move into z_all via DMA
                    nc.sync.dma_start(out=z_all[16 + 16 * j:32 + 16 * j, :],
                                      in_=z_s[:])

                # scatter owned rows into ag_in (dest rows from pidx;
                # rows with pidx > 53 are dropped)
                nc.gpsimd.indirect_dma_start(
                    out=ag_in[:, :],
                    out_offset=bass.IndirectOffsetOnAxis(
                        ap=pidx_sb[:, 0:1], axis=0),
                    in_=z_all[:, :], in_offset=None,
                    bounds_check=NIL - 1, oob_is_err=False)

            nc.gpsimd.collective_compute(
                "AllGather", mybir.AluOpType.bypass, replica_groups=rg,
                ins=[ag_in[:].opt()], outs=[ag_out[:].opt()])

            # ---- post-AG: ziT gather, b_all broadcast, j-sources ----
            ziT = cp.tile([NIL, BATCH], dt.bfloat16, tag="ziT", name="ziT")
            nc.gpsimd.indirect_dma_start(
                out=ziT[:], out_offset=None, in_=ag_out[:, :],
                in_offset=bass.IndirectOffsetOnAxis(ap=zidx_sb[:, 0:1], axis=0))
            nc.sync.dma_start(out=zi_d[:, :], in_=ziT[:])

            b_all = cp.tile([128, NIL, BATCH], dt.bfloat16, tag="b_all",
                            name="b_all")
            GRP = 7
            gi = 0
            for g0 in range(0, NIL, GRP):
                g1 = min(NIL, g0 + GRP)
                ring = nc.sync if gi % 2 == 0 else nc.scalar
                ring.dma_start(
                    out=b_all[:, g0:g1, :],
                    in_=zi_d[g0:g1, :].unsqueeze(0).to_broadcast(
                        [128, g1 - g0, BATCH]))
                gi += 1

            ztf = cp.tile([128, 3, BATCH], dt.bfloat16, tag="ztf", name="ztf")
            for jc in range(3):
                nc.sync.dma_start(out=ztf[:, jc, :],
                                  in_=ag_out[128 * jc:128 * (jc + 1), :])
            t3 = cp.tile([128, 2, BATCH], dt.bfloat16, tag="t3", name="t3")
            # col 0 (t3a): 384 + (p % 48)
            nc.scalar.dma_start(out=t3[0:48, 0, :], in_=ag_out[384:432, :])
            nc.scalar.dma_start(out=t3[48:96, 0, :], in_=ag_out[384:432, :])
            nc.scalar.dma_start(out=t3[96:128, 0, :], in_=ag_out[384:416, :])
            # col 1 (t3b): 384 + ((p+24) % 48)
            nc.scalar.dma_start(out=t3[0:24, 1, :], in_=ag_out[408:432, :])
            nc.scalar.dma_start(out=t3[24:72, 1, :], in_=ag_out[384:432, :])
            nc.scalar.dma_start(out=t3[72:120, 1, :], in_=ag_out[384:432, :])
            nc.scalar.dma_start(out=t3[120:128, 1, :], in_=ag_out[384:392, :])

            def src_ap(src, p0, plen):
                if src.startswith("zt"):
                    return ztf[p0:p0 + plen, int(src[2]), :]
                col = 0 if src == "t3a" else 1
                return t3[p0:p0 + plen, col, :]

            # ---- main loop ----
            with tc.tile_pool(name="ps_acc", bufs=1, space="PSUM") as pa, \
                 tc.tile_pool(name="outp", bufs=1) as op_:

                acc = [pa.tile([128, BATCH], dt.float32, tag=f"acc{oc}",
                               name=f"acc{oc}") for oc in range(4)]

                def aligned(ops):
                    """Decompose segments into legal partition spans.

                    Engine partition access must start at base 0/32/64/96
                    (base 32 may only span to 64). Rounded-down pieces
                    write garbage below their true start; emitting in
                    descending true-start order makes each row's final
                    writer its owner (untouched garbage rows have zero
                    weights).
                    """
                    pieces = []
                    for (p0, plen, src, il) in ops:
                        p1, cur = p0 + plen, p0
                        while cur < p1:
                            a = 32 * (cur // 32)
                            end = 64 if a == 32 else 128
                            take = min(p1, end)
                            pieces.append((a, take, src, il, cur))
                            cur = take
                    pieces.sort(key=lambda x: -x[4])
                    return pieces

                segctr = 0
                for db in range(NDBL):
                    itb = itbs[db % NBUF]
                    # 4 chunks -> slots 0..3 of the double-tile
                    for s4 in range(4):
                        t = 4 * db + s4
                        for (a, take, src, il, _tr) in aligned(CHUNKS[t]):
                            eng = nc.gpsimd if segctr % 6 == 5 else nc.vector
                            eng.tensor_mul(
                                out=itb[a:take, s4, :],
                                in0=src_ap(src, a, take - a),
                                in1=b_all[a:take, il, :])
                            segctr += 1
                    it = ip.tile([128, 4, BATCH], dt.float8e4, tag="it",
                                 name="it")
                    if db % 4 == 3:
                        nc.gpsimd.tensor_copy(out=it[:, :, :], in_=itb[:, :, :])
                    else:
                        nc.scalar.activation(
                            out=it[:, :, :], in_=itb[:, :, :],
                            func=mybir.ActivationFunctionType.Copy)
                    for half in range(2):
                        pr = 2 * db + half
                        for oc in range(4):
                            nc.tensor.matmul(
                                out=acc[oc][:],
                                lhsT=wsb[:, 0:2,
                                         pr * O + oc * 128:pr * O + (oc + 1) * 128],
                                rhs=it[:, 2 * half:2 * half + 2, :],
                                start=(pr == 0), stop=(pr == NPAIR - 1),
                                perf_mode=mybir.MatmulPerfMode.DoubleRow)

                for oc in range(4):
                    osb = op_.tile([128, BATCH], dt.bfloat16, tag=f"osb{oc}",
                                   name=f"osb{oc}")
                    if oc % 2 == 0:
                        nc.vector.tensor_scalar_mul(out=osb[:], in0=acc[oc][:],
                                                    scalar1=1.0 / 256.0)
                        nc.sync.dma_start(out=ar_in[oc * 128:(oc + 1) * 128, :],
                                          in_=osb[:])
                    else:
                        nc.scalar.activation(
                            out=osb[:], in_=acc[oc][:],
                            func=mybir.ActivationFunctionType.Copy,
                            scale=1.0 / 256.0)
                        nc.scalar.dma_start(out=ar_in[oc * 128:(oc + 1) * 128, :],
                                            in_=osb[:])

            nc.gpsimd.collective_compute(
                "AllReduce", mybir.AluOpType.add, replica_groups=rg,
                ins=[ar_in[:].opt()], outs=[ar_out[:].opt()])

            # ---- prediction MLP tail (no scalar relus: sigmoid table
            # stays resident) ----
            with tc.tile_pool(name="tail_sb", bufs=1) as ts, \
                 tc.tile_pool(name="ps_t", bufs=2, space="PSUM") as pt:
                h0 = []
                for kc in range(4):
                    r = ts.tile([128, BATCH], dt.bfloat16, tag=f"red{kc}",
                                name=f"red{kc}")
                    ring = nc.sync if kc % 2 == 0 else nc.scalar
                    ring.dma_start(out=r[:], in_=ar_out[kc * 128:(kc + 1) * 128, :])
                    h = ts.tile([128, BATCH], dt.bfloat16, tag=f"h0_{kc}",
                                name=f"h0_{kc}")
                    eng = nc.vector if kc % 2 == 0 else nc.gpsimd
                    eng.tensor_scalar(
                        out=h[:], in0=r[:], scalar1=pb0_sb[:, kc:kc + 1],
                        scalar2=0.0, op0=mybir.AluOpType.add,
                        op1=mybir.AluOpType.max)
                    h0.append(h)
                h1p = []
                for mc in range(2):
                    p1 = pt.tile([128, BATCH], dt.float32, tag=f"p1_{mc}",
                                 name=f"p1_{mc}")
                    for kc in range(4):
                        nc.tensor.matmul(out=p1[:],
                                         lhsT=pw1_sb[kc][:, mc * 128:(mc + 1) * 128],
                                         rhs=h0[kc][:], start=(kc == 0), stop=(kc == 3))
                    h = ts.tile([128, BATCH], dt.bfloat16, tag=f"h1p_{mc}",
                                name=f"h1p_{mc}")
                    eng = nc.vector  # gpsimd cannot read PSUM
                    eng.tensor_scalar(
                        out=h[:], in0=p1[:], scalar1=pb1_sb[:, mc:mc + 1],
                        scalar2=0.0, op0=mybir.AluOpType.add,
                        op1=mybir.AluOpType.max)
                    h1p.append(h)
                p2 = pt.tile([1, BATCH], dt.float32, tag="p2", name="p2")
                for mc in range(2):
                    nc.tensor.matmul(out=p2[:], lhsT=pw2_sb[mc][:, :], rhs=h1p[mc][:],
                                     start=(mc == 0), stop=(mc == 1))
                res = ts.tile([1, BATCH], dt.float32, tag="res", name="res")
                nc.scalar.activation(out=res[:], in_=p2[:],
                                     func=mybir.ActivationFunctionType.Sigmoid,
                                     bias=pb2_sb[:, 0:1])
                nc.sync.dma_start(out=out_d[:, :], in_=res[:])

    nc.compile()
    return nc


def _host_prep(inputs):
    f32 = np.float32
    df = np.asarray(inputs["dense_features"], f32)
    sf = np.asarray(inputs["sparse_features"])
    emb = np.asarray(inputs["emb"], f32)
    pw0 = np.asarray(inputs["pw0"], f32)

    idx = ((sf.astype(np.int64) + 1) % CARD).astype(np.int32)   # [512, 26]
    embb = emb.astype(BF16)                                     # [26, CARD, 16]

    pw0v = pw0.reshape(ZR, ZR, O)
    Wfull = pw0v + pw0v.transpose(1, 0, 2)
    ar = np.arange(ZR)
    Wfull[ar, ar] = pw0v[ar, ar]
    Wb = Wfull.astype(BF16)                                     # [432, 432, 512]
    del Wfull

    dfT = np.zeros((16, BATCH), BF16)
    dfT[:13] = df.T.astype(BF16)
    dw0p = np.zeros((16, 512), f32)
    dw0p[:13] = np.asarray(inputs["dw0"], f32)

    def col(b, p):
        return np.asarray(b, f32).reshape(p, 128).T.copy()

    common = {
        "dfT": dfT,
        "dw0": dw0p.astype(BF16),
        "dw1": np.asarray(inputs["dw1"], f32).astype(BF16),
        "dw2": np.asarray(inputs["dw2"], f32).astype(BF16),
        "db0q": col(inputs["db0"], 4),
        "db1q": col(inputs["db1"], 2),
        "db2q": np.asarray(inputs["db2"], f32).reshape(64, 1).copy(),
        "pw1": np.asarray(inputs["pw1"], f32).astype(BF16),
        "pw2": np.asarray(inputs["pw2"], f32).reshape(256, 1).astype(BF16),
        "pb0q": col(inputs["pb0"], 4),
        "pb1q": col(inputs["pb1"], 2),
        "pb2q": np.asarray(inputs["pb2"], f32).reshape(1, 1).copy(),
    }
    dw3 = np.asarray(inputs["dw3"], f32).astype(BF16)
    db3 = np.asarray(inputs["db3"], f32).reshape(16, 1).astype(f32)
    zero_tab = np.zeros((CARD, ED), BF16)
    zero_idx = np.zeros(BATCH, np.int32)

    Jt = JMAP.reshape(-1)            # [NCH*128], -1 = gap
    ILt = ILMAP.reshape(-1)
    JSt = np.array([_jstart(il) for il in range(NIL)], np.int64)[ILt]

    in_maps = []
    for c in range(NCORES):
        m = dict(common)
        m["dw3"] = dw3 if c == 0 else np.zeros_like(dw3)
        m["db3q"] = db3 if c == 0 else np.zeros_like(db3)

        I = JSt + c
        Wc = Wb[I, np.maximum(Jt, 0)]        # [NCH*128, 512] bf16
        Wc[(Jt < 0) | (Jt < I)] = 0
        Wq = (Wc.astype(f32) * 256.0).astype(F8)
        m["wsb"] = np.ascontiguousarray(
            Wq.reshape(NPAIR, 2, 128, O).transpose(2, 1, 0, 3)
            .reshape(128, 2, NPAIR * O))

        m["zidx"] = np.array([[_i_of(il, c)] for il in range(NIL)], np.int32)

        # tables touched by rows [54c, 54c+54)
        units = sorted({r // 16 for r in range(54 * c, 54 * c + 54)})
        tabs = [u - 1 for u in units if u > 0]   # 0-based table indices
        assert len(tabs) <= 5
        pidx_ = np.full((96, 1), 1000, np.int32)
        if c == 0:
            pidx_[0:16, 0] = np.arange(16)
        iq = np.zeros((128, 20), np.int32)
        for j in range(5):
            if j < len(tabs):
                tj = tabs[j]
                m[f"es{j}"] = np.ascontiguousarray(embb[tj])
                icol = idx[:, tj]
                zr0 = 16 * (tj + 1)          # true z row of dim 0
                for d in range(16):
                    agr = zr0 + d - 54 * c
                    if 0 <= agr < NIL:
                        pidx_[16 + 16 * j + d, 0] = agr
            else:
                m[f"es{j}"] = zero_tab
                icol = zero_idx
            iq[:, j * 4:(j + 1) * 4] = icol.reshape(4, 128).T
        m["idxq"] = iq
        m["pidx"] = pidx_
        in_maps.append(m)
    return in_maps


def kernel(**inputs):
    from concourse import bass_utils
    import os

    if "nc" not in _state:
        _state["nc"] = _build_module()
    in_maps = _host_prep(inputs)
    trace = bool(int(os.environ.get("DLRM_TRACE", "0")))
    res = bass_utils.run_bass_kernel_spmd(
        _state["nc"], in_maps, core_ids=list(range(NCORES)), trace=trace)
    _state["last_results"] = res
    return np.asarray(res.results[0]["out"], np.float32).reshape(BATCH)


# revision 13
# speedup vs baseline: 1.2060x; 1.2060x over previous
"""DLRM forward on 8 Trainium2 NeuronCores (Bass/Tile).

Strategy (v5):
- True-z-order sharding: core m owns z rows [54m, 54m+54) (z = dense(16)
  | 26 tables x 16). Each core gathers its <=4 tables (4 indirect DMAs
  per table, one per 128-batch block), PE-transposes to [16, 512], and
  scatters the rows it owns into ag_in via one indirect DMA. AllGather
  then yields ag_out = full z in TRUE row order, so the interaction
  j-sources are plain strided DMAs (no post-AG gathers except ziT).
- Interaction + pred layer 0 use host-folded symmetric weights in fp8:
  out[o,b] = sum_{i<=j} Wf[(i,j),o] z_i[b] z_j[b]. Core c owns i-rows
  {128q+8d+c} u {384+8d+c}. 100 exact-packed 128-row chunks (7% pad),
  50 DoubleRow K=256 pairs, 4 PSUM banks. DVE builds bf16 products
  (mostly vector engine, 2x mode); scalar/gpsimd convert to fp8 in
  2-pair batches; PE runs back-to-back DoubleRow matmuls (HAM-warm).
- Partial out^T is AllReduced (bf16); every core computes the pred tail
  (relu on vector/gpsimd, sigmoid table preloaded) redundantly.
"""

import numpy as np
import ml_dtypes

BATCH = 512
CARD = 100000
ED = 16
NCORES = 8
NIL = 54
ZR = 432
O = 512

BF16 = ml_dtypes.bfloat16
F8 = ml_dtypes.float8_e4m3fn

_state = {}


def _jstart(il):
    if il < 48:
        q, dd = divmod(il, 16)
        return 128 * q + 8 * dd
    return 384 + 8 * (il - 48)


def _i_of(il, c):
    return _jstart(il) + c


def _src_j(src, p):
    if src.startswith("zt"):
        return 128 * int(src[2]) + p
    if src == "t3a":
        return 384 + (p % 48)
    return 384 + ((p + 24) % 48)  # t3b


def _build_recipe():
    """100 chunks (50 pairs): see recipe analysis.

    Returns (chunks, jmap, ilmap): chunks = list of segment lists
    (p0, plen, src, il); jmap/ilmap [100,128], jmap=-1 where unused.
    """
    pieces = [(il, 0, 48) for il in range(49)]
    pieces += [(48 + d, 8 * d, 48 - 8 * d) for d in range(1, 6)]

    diag_windows = []
    for dd in range(16):
        for q in range(3):
            for k in range(8 * dd // 48):
                diag_windows.append((q, dd, 48 * k))
    diag_windows.sort(key=lambda w: (w[1], w[0], w[2]))
    whole = [p for p in pieces if p[2] == 48]
    rest = [p for p in pieces if p[2] < 48]
    gap_fill = {}
    for (q, dd, p0), (il, lo, ln) in zip(diag_windows, whole[:42]):
        gap_fill.setdefault((q, dd), []).append((p0, 48, "t3a", il))
    leftover = whole[42:] + rest

    tail_chunks = [[] for _ in range(4)]
    slots = []
    for ci in range(4):
        slots += [(ci, 0), (ci, 48)]
    part_ps = [p for p in leftover if p[1] > 0]
    full_ps = [p for p in leftover if p[1] == 0]
    fi = 0
    for (ci, w0) in slots[:7]:
        il, lo, ln = full_ps[fi]
        tail_chunks[ci].append((w0, 48, "t3a", il))
        fi += 1
    il, lo, ln = part_ps[0]
    tail_chunks[3].append((48 + lo, ln, "t3a", il))

    part_regions = []
    for dd in range(16):
        L = 8 * dd % 48
        if L:
            for q in range(3):
                part_regions.append((q, dd, 48 * (8 * dd // 48), L))
    part_regions.sort(key=lambda r: -r[3])
    atoms = {}
    for (il2, lo2, ln2) in part_ps[1:]:
        atoms[il2] = list(range(lo2, 48))
    for (q, dd, p0, L) in part_regions:
        if not atoms:
            break
        cands = [a for a, rs in atoms.items() if rs and rs[0] < L]
        if not cands:
            continue
        a = min(cands, key=lambda x: atoms[x][0])
        rs = atoms[a]
        take = [r for r in rs if r < L]
        if not take:
            continue
        lo3, hi3 = take[0], take[-1]
        gap_fill.setdefault((q, dd), []).append(
            (p0 + lo3, hi3 + 1 - lo3, "t3a", a))
        atoms[a] = [r for r in rs if r >= L]
        if not atoms[a]:
            del atoms[a]
    for ci, a in enumerate(sorted(atoms)):
        tail_chunks[ci].append((112, 8, "t3b", a))

    chunks = []
    for il in range(16):
        chunks.append([(0, 128, "zt1", il)])
        chunks.append([(0, 128, "zt2", il)])
    for il in range(16, 32):
        chunks.append([(0, 128, "zt2", il)])
    for dd in range(16):
        for q in range(3):
            il = 16 * q + dd
            ops = [(8 * dd, 128 - 8 * dd, f"zt{q}", il)]
            ops += gap_fill.get((q, dd), [])
            ops.sort()
            chunks.append(ops)
    for ops in tail_chunks:
        ops.sort()
        chunks.append(ops)

    nch = len(chunks)
    jmap = -np.ones((nch, 128), np.int64)
    ilmap = np.zeros((nch, 128), np.int64)
    for t, ops in enumerate(chunks):
        for (p0, plen, src, il4) in ops:
            for e in range(plen):
                p = p0 + e
                jmap[t, p] = _src_j(src, p)
                ilmap[t, p] = il4
    return chunks, jmap, ilmap


CHUNKS, JMAP, ILMAP = _build_recipe()
NCH = len(CHUNKS)
NPAIR = NCH // 2
NDBL = NPAIR // 2   # double-tiles of 4 chunks
NBUF = 5            # ring of double-tiles


def _build_module():
    import concourse.bass as bass
    import concourse.mybir as mybir
    import concourse.tile as tile
    from concourse import bacc
    from concourse.masks import make_identity

    dt = mybir.dt
    nc = bacc.Bacc("TRN2", target_bir_lowering=False, debug=False,
                   num_devices=NCORES)

    wsb_d = nc.dram_tensor("wsb", [128, 2, NPAIR * O], dt.float8e4,
                           kind="ExternalInput").ap()
    zidx = nc.dram_tensor("zidx", [NIL, 1], dt.int32, kind="ExternalInput").ap()
    pidx = nc.dram_tensor("pidx", [96, 1], dt.int32, kind="ExternalInput").ap()
    embs = {}
    for j in range(5):
        embs[j] = nc.dram_tensor(f"es{j}", [CARD, ED], dt.bfloat16,
                                 kind="ExternalInput").ap()
    idxq = nc.dram_tensor("idxq", [128, 20], dt.int32, kind="ExternalInput").ap()
    dfT = nc.dram_tensor("dfT", [16, BATCH], dt.bfloat16, kind="ExternalInput").ap()
    dw0 = nc.dram_tensor("dw0", [16, 512], dt.bfloat16, kind="ExternalInput").ap()
    dw1 = nc.dram_tensor("dw1", [512, 256], dt.bfloat16, kind="ExternalInput").ap()
    dw2 = nc.dram_tensor("dw2", [256, 64], dt.bfloat16, kind="ExternalInput").ap()
    dw3 = nc.dram_tensor("dw3", [64, 16], dt.bfloat16, kind="ExternalInput").ap()
    db0q = nc.dram_tensor("db0q", [128, 4], dt.float32, kind="ExternalInput").ap()
    db1q = nc.dram_tensor("db1q", [128, 2], dt.float32, kind="ExternalInput").ap()
    db2q = nc.dram_tensor("db2q", [64, 1], dt.float32, kind="ExternalInput").ap()
    db3q = nc.dram_tensor("db3q", [16, 1], dt.float32, kind="ExternalInput").ap()
    pw1 = nc.dram_tensor("pw1", [512, 256], dt.bfloat16, kind="ExternalInput").ap()
    pw2 = nc.dram_tensor("pw2", [256, 1], dt.bfloat16, kind="ExternalInput").ap()
    pb0q = nc.dram_tensor("pb0q", [128, 4], dt.float32, kind="ExternalInput").ap()
    pb1q = nc.dram_tensor("pb1q", [128, 2], dt.float32, kind="ExternalInput").ap()
    pb2q = nc.dram_tensor("pb2q", [1, 1], dt.float32, kind="ExternalInput").ap()
    out_d = nc.dram_tensor("out", [1, BATCH], dt.float32, kind="ExternalOutput").ap()

    rg = [list(range(NCORES))]

    with tile.TileContext(nc) as tc:
        with tc.tile_pool(name="const", bufs=1) as cp, \
             tc.tile_pool(name="ip", bufs=6) as ip, \
             tc.tile_pool(name="dram", bufs=1, space="DRAM") as dp:

            ag_in = dp.tile([NIL, BATCH], dt.bfloat16, tag="ag_in", name="ag_in")
            ag_out = dp.tile([ZR, BATCH], dt.bfloat16, tag="ag_out", name="ag_out")
            zi_d = dp.tile([NIL, BATCH], dt.bfloat16, tag="zi_d", name="zi_d")
            ar_in = dp.tile([O, BATCH], dt.bfloat16, tag="ar_in", name="ar_in")
            ar_out = dp.tile([O, BATCH], dt.bfloat16, tag="ar_out", name="ar_out")

            wsb = cp.tile([128, 2, NPAIR * O], dt.float8e4, tag="wsb", name="wsb")

            ident = cp.tile([128, 128], dt.bfloat16, tag="ident", name="ident")
            make_identity(nc, ident[:])
            idx_sb = cp.tile([128, 20], dt.int32, tag="idx_sb", name="idx_sb")
            nc.sync.dma_start(out=idx_sb[:], in_=idxq[:, :])
            zidx_sb = cp.tile([NIL, 1], dt.int32, tag="zidx_sb", name="zidx_sb")
            nc.sync.dma_start(out=zidx_sb[:], in_=zidx[:, :])
            pidx_sb = cp.tile([96, 1], dt.int32, tag="pidx_sb", name="pidx_sb")
            nc.sync.dma_start(out=pidx_sb[:], in_=pidx[:, :])
            dfT_sb = cp.tile([16, BATCH], dt.bfloat16, tag="dfT_sb", name="dfT_sb")
            nc.sync.dma_start(out=dfT_sb[:], in_=dfT[:, :])
            dw0_sb = cp.tile([16, 512], dt.bfloat16, tag="dw0_sb", name="dw0_sb")
            nc.sync.dma_start(out=dw0_sb[:], in_=dw0[:, :])
            dw1_sb = [cp.tile([128, 256], dt.bfloat16, tag=f"dw1_{k}", name=f"dw1_{k}")
                      for k in range(4)]
            for k in range(4):
                nc.sync.dma_start(out=dw1_sb[k][:], in_=dw1[k * 128:(k + 1) * 128, :])
            dw2_sb = [cp.tile([128, 64], dt.bfloat16, tag=f"dw2_{k}", name=f"dw2_{k}")
                      for k in range(2)]
            for k in range(2):
                nc.sync.dma_start(out=dw2_sb[k][:], in_=dw2[k * 128:(k + 1) * 128, :])
            dw3_sb = cp.tile([64, 16], dt.bfloat16, tag="dw3_sb", name="dw3_sb")
            nc.sync.dma_start(out=dw3_sb[:], in_=dw3[:, :])
            pw1_sb = [cp.tile([128, 256], dt.bfloat16, tag=f"pw1_{k}", name=f"pw1_{k}")
                      for k in range(4)]
            for k in range(4):
                nc.sync.dma_start(out=pw1_sb[k][:], in_=pw1[k * 128:(k + 1) * 128, :])
            pw2_sb = [cp.tile([128, 1], dt.bfloat16, tag=f"pw2_{k}", name=f"pw2_{k}")
                      for k in range(2)]
            for k in range(2):
                nc.sync.dma_start(out=pw2_sb[k][:], in_=pw2[k * 128:(k + 1) * 128, :])
            db0_sb = cp.tile([128, 4], dt.float32, tag="db0_sb", name="db0_sb")
            nc.sync.dma_start(out=db0_sb[:], in_=db0q[:, :])
            db1_sb = cp.tile([128, 2], dt.float32, tag="db1_sb", name="db1_sb")
            nc.sync.dma_start(out=db1_sb[:], in_=db1q[:, :])
            db2_sb = cp.tile([64, 1], dt.float32, tag="db2_sb", name="db2_sb")
            nc.sync.dma_start(out=db2_sb[:], in_=db2q[:, :])
            db3_sb = cp.tile([16, 1], dt.float32, tag="db3_sb", name="db3_sb")
            nc.sync.dma_start(out=db3_sb[:], in_=db3q[:, :])
            pb0_sb = cp.tile([128, 4], dt.float32, tag="pb0_sb", name="pb0_sb")
            nc.sync.dma_start(out=pb0_sb[:], in_=pb0q[:, :])
            pb1_sb = cp.tile([128, 2], dt.float32, tag="pb1_sb", name="pb1_sb")
            nc.sync.dma_start(out=pb1_sb[:], in_=pb1q[:, :])
            pb2_sb = cp.tile([1, 1], dt.float32, tag="pb2_sb", name="pb2_sb")
            nc.sync.dma_start(out=pb2_sb[:], in_=pb2q[:, :])



            # itb double-tile ring; pre-zero once (gap rows stay finite;
            # fp8 'it' gap rows hit zero weights so stale data is fine)
            itbs = [ip.tile([128, 4, BATCH], dt.bfloat16, tag="itb",
                            name="itb") for _ in range(NBUF)]
            for t_ in itbs:
                nc.vector.memset(t_[:, :, :], 0)

            # ---- dense MLP + gathers + scatter ----
            with tc.tile_pool(name="gather", bufs=2) as gp, \
                 tc.tile_pool(name="ps_g", bufs=2, space="PSUM") as pg, \
                 tc.tile_pool(name="ps_d", bufs=4, space="PSUM") as pd, \
                 tc.tile_pool(name="dmlp", bufs=1) as dm:
                h1 = []
                for mc in range(4):
                    d1 = pd.tile([128, BATCH], dt.float32, tag="dscr", name="d1")
                    nc.tensor.matmul(out=d1[:], lhsT=dw0_sb[:, mc * 128:(mc + 1) * 128],
                                     rhs=dfT_sb[:], start=True, stop=True)
                    h = dm.tile([128, BATCH], dt.bfloat16, tag=f"h1_{mc}",
                                name=f"h1_{mc}")
                    nc.scalar.activation(out=h[:], in_=d1[:],
                                         func=mybir.ActivationFunctionType.Relu,
                                         bias=db0_sb[:, mc:mc + 1])
                    h1.append(h)
                h2 = []
                for mc in range(2):
                    d2 = pd.tile([128, BATCH], dt.float32, tag="dscr", name="d2")
                    for k in range(4):
                        nc.tensor.matmul(out=d2[:],
                                         lhsT=dw1_sb[k][:, mc * 128:(mc + 1) * 128],
                                         rhs=h1[k][:], start=(k == 0), stop=(k == 3))
                    h = dm.tile([128, BATCH], dt.bfloat16, tag=f"h2_{mc}",
                                name=f"h2_{mc}")
                    nc.scalar.activation(out=h[:], in_=d2[:],
                                         func=mybir.ActivationFunctionType.Relu,
                                         bias=db1_sb[:, mc:mc + 1])
                    h2.append(h)
                d3 = pd.tile([128, BATCH], dt.float32, tag="dscr", name="d3")
                for k in range(2):
                    nc.tensor.matmul(out=d3[:64, :], lhsT=dw2_sb[k][:, :],
                                     rhs=h2[k][:], start=(k == 0), stop=(k == 1))
                h3 = dm.tile([64, BATCH], dt.bfloat16, tag="h3", name="h3")
                nc.scalar.activation(out=h3[:], in_=d3[:64, :],
                                     func=mybir.ActivationFunctionType.Relu,
                                     bias=db2_sb[:, 0:1])
                d4 = pd.tile([128, BATCH], dt.float32, tag="dscr", name="d4")
                nc.tensor.matmul(out=d4[:16, :], lhsT=dw3_sb[:, :], rhs=h3[:],
                                 start=True, stop=True)

                # z_all: rows 0-15 dense, 16+16j table slot j
                z_all = cp.tile([96, BATCH], dt.bfloat16, tag="z_all",
                                name="z_all")
                nc.vector.tensor_scalar_add(out=z_all[0:16, :], in0=d4[:16, :],
                                            scalar1=db3_sb[:, 0:1])

                # preload sigmoid ACT table while scalar is idle (dense
                # relus done; loop converts use Copy which needs no set)
                sig_scr = dm.tile([1, 1], dt.float32, tag="sig_scr",
                                  name="sig_scr")
                nc.scalar.activation(out=sig_scr[:], in_=pb2_sb[:, :],
                                     func=mybir.ActivationFunctionType.Sigmoid)
                # weight slab prefetch: one big DMA on the scalar ring,
                # emitted after the dense relus so its ring-drain stall
                # doesn't delay them
                nc.scalar.dma_start(out=wsb[:, :, :], in_=wsb_d[:, :, :])

                for j in range(5):
                    ps_s = pg.tile([16, BATCH], dt.bfloat16, tag="psg",
                                   name=f"psg{j}")
                    for bc in range(4):
                        gt = gp.tile([128, ED], dt.bfloat16, tag=f"g{j}_{bc}",
                                     name=f"g{j}_{bc}")
                        nc.gpsimd.indirect_dma_start(
                            out=gt[:], out_offset=None, in_=embs[j][:, :],
                            in_offset=bass.IndirectOffsetOnAxis(
                                ap=idx_sb[:, j * 4 + bc:j * 4 + bc + 1], axis=0))
                        nc.tensor.transpose(out=ps_s[:16, bc * 128:(bc + 1) * 128],
                                            in_=gt[:, :], identity=ident[:])
                    z_s = dm.tile([16, BATCH], dt.bfloat16, tag=f"zsl{j}",
                                  name=f"zsl{j}")
                    if j % 2 == 0:
                        nc.vector.tensor_copy(out=z_s[:], in_=ps_s[:16, :])
                    else:
                        nc.scalar.activation(
                            out=z_s[:], in_=ps_s[:16, :],
                            func=mybir.ActivationFunctionType.Copy)
                    # partition-shifting move into z_all via DMA
                    nc.sync.dma_start(out=z_all[16 + 16 * j:32 + 16 * j, :],
                                      in_=z_s[:])

                # scatter owned rows into ag_in (dest rows from pidx;
                # rows with pidx > 53 are dropped)
                nc.gpsimd.indirect_dma_start(
                    out=ag_in[:, :],
                    out_offset=bass.IndirectOffsetOnAxis(
                        ap=pidx_sb[:, 0:1], axis=0),
                    in_=z_all[:, :], in_offset=None,
                    bounds_check=NIL - 1, oob_is_err=False)

            nc.gpsimd.collective_compute(
                "AllGather", mybir.AluOpType.bypass, replica_groups=rg,
                ins=[ag_in[:].opt()], outs=[ag_out[:].opt()])

            # ---- post-AG: ziT gather, b_all broadcast, j-sources ----
            ziT = cp.tile([NIL, BATCH], dt.bfloat16, tag="ziT", name="ziT")
            nc.gpsimd.indirect_dma_start(
                out=ziT[:], out_offset=None, in_=ag_out[:, :],
                in_offset=bass.IndirectOffsetOnAxis(ap=zidx_sb[:, 0:1], axis=0))
            nc.sync.dma_start(out=zi_d[:, :], in_=ziT[:])

            b_all = cp.tile([128, NIL, BATCH], dt.bfloat16, tag="b_all",
                            name="b_all")
            GRP = 7
            gi = 0
            for g0 in range(0, NIL, GRP):
                g1 = min(NIL, g0 + GRP)
                ring = nc.sync if gi % 2 == 0 else nc.scalar
                ring.dma_start(
                    out=b_all[:, g0:g1, :],
                    in_=zi_d[g0:g1, :].unsqueeze(0).to_broadcast(
                        [128, g1 - g0, BATCH]))
                gi += 1

            ztf = cp.tile([128, 3, BATCH], dt.bfloat16, tag="ztf", name="ztf")
            for jc in range(3):
                nc.sync.dma_start(out=ztf[:, jc, :],
                                  in_=ag_out[128 * jc:128 * (jc + 1), :])
            t3 = cp.tile([128, 2, BATCH], dt.bfloat16, tag="t3", name="t3")
            # col 0 (t3a): 384 + (p % 48)
            nc.scalar.dma_start(out=t3[0:48, 0, :], in_=ag_out[384:432, :])
            nc.scalar.dma_start(out=t3[48:96, 0, :], in_=ag_out[384:432, :])
            nc.scalar.dma_start(out=t3[96:128, 0, :], in_=ag_out[384:416, :])
            # col 1 (t3b): 384 + ((p+24) % 48)
            nc.scalar.dma_start(out=t3[0:24, 1, :], in_=ag_out[408:432, :])
            nc.scalar.dma_start(out=t3[24:72, 1, :], in_=ag_out[384:432, :])
            nc.scalar.dma_start(out=t3[72:120, 1, :], in_=ag_out[384:432, :])
            nc.scalar.dma_start(out=t3[120:128, 1, :], in_=ag_out[384:392, :])

            def src_ap(src, p0, plen):
                if src.startswith("zt"):
                    return ztf[p0:p0 + plen, int(src[2]), :]
                col = 0 if src == "t3a" else 1
                return t3[p0:p0 + plen, col, :]

            # ---- main loop ----
            with tc.tile_pool(name="ps_acc", bufs=1, space="PSUM") as pa, \
                 tc.tile_pool(name="outp", bufs=1) as op_:

                acc = [pa.tile([128, BATCH], dt.float32, tag=f"acc{oc}",
                               name=f"acc{oc}") for oc in range(4)]

                def aligned(ops):
                    """Decompose segments into legal partition spans.

                    Engine partition access must start at base 0/32/64/96
                    (base 32 may only span to 64). Rounded-down pieces
                    write garbage below their true start; emitting in
                    descending true-start order makes each row's final
                    writer its owner (untouched garbage rows have zero
                    weights).
                    """
                    pieces = []
                    for (p0, plen, src, il) in ops:
                        p1, cur = p0 + plen, p0
                        while cur < p1:
                            a = 32 * (cur // 32)
                            end = 64 if a == 32 else 128
                            take = min(p1, end)
                            pieces.append((a, take, src, il, cur))
                            cur = take
                    pieces.sort(key=lambda x: -x[4])
                    return pieces

                segctr = 0
                for db in range(NDBL):
                    itb = itbs[db % NBUF]
                    # 4 chunks -> slots 0..3 of the double-tile
                    for s4 in range(4):
                        t = 4 * db + s4
                        for (a, take, src, il, _tr) in aligned(CHUNKS[t]):
                            eng = nc.gpsimd if segctr % 6 == 5 else nc.vector
                            eng.tensor_mul(
                                out=itb[a:take, s4, :],
                                in0=src_ap(src, a, take - a),
                                in1=b_all[a:take, il, :])
                            segctr += 1
                    it = ip.tile([128, 4, BATCH], dt.float8e4, tag="it",
                                 name="it")
                    nc.scalar.activation(
                        out=it[:, :, :], in_=itb[:, :, :],
                        func=mybir.ActivationFunctionType.Copy)
                    for half in range(2):
                        pr = 2 * db + half
                        for oc in range(4):
                            nc.tensor.matmul(
                                out=acc[oc][:],
                                lhsT=wsb[:, 0:2,
                                         pr * O + oc * 128:pr * O + (oc + 1) * 128],
                                rhs=it[:, 2 * half:2 * half + 2, :],
                                start=(pr == 0), stop=(pr == NPAIR - 1),
                                perf_mode=mybir.MatmulPerfMode.DoubleRow)

                for oc in range(4):
                    osb = op_.tile([128, BATCH], dt.bfloat16, tag=f"osb{oc}",
                                   name=f"osb{oc}")
                    if oc % 2 == 0:
                        nc.vector.tensor_scalar_mul(out=osb[:], in0=acc[oc][:],
                                                    scalar1=1.0 / 256.0)
                        nc.sync.dma_start(out=ar_in[oc * 128:(oc + 1) * 128, :],
                                          in_=osb[:])
                    else:
                        nc.scalar.activation(
                            out=osb[:], in_=acc[oc][:],
                            func=mybir.ActivationFunctionType.Copy,
                            scale=1.0 / 256.0)
                        nc.scalar.dma_start(out=ar_in[oc * 128:(oc + 1) * 128, :],
                                            in_=osb[:])

            nc.gpsimd.collective_compute(
                "AllReduce", mybir.AluOpType.add, replica_groups=rg,
                ins=[ar_in[:].opt()], outs=[ar_out[:].opt()])

            # ---- prediction MLP tail (no scalar relus: sigmoid table
            # stays resident) ----
            with tc.tile_pool(name="tail_sb", bufs=1) as ts, \
                 tc.tile_pool(name="ps_t", bufs=2, space="PSUM") as pt:
                h0 = []
                for kc in range(4):
                    r = ts.tile([128, BATCH], dt.bfloat16, tag=f"red{kc}",
                                name=f"red{kc}")
                    ring = nc.sync if kc % 2 == 0 else nc.scalar
                    ring.dma_start(out=r[:], in_=ar_out[kc * 128:(kc + 1) * 128, :])
                    h = ts.tile([128, BATCH], dt.bfloat16, tag=f"h0_{kc}",
                                name=f"h0_{kc}")
                    eng = nc.vector  # gpsimd tensor_scalar is ~7.5us
                    eng.tensor_scalar(
                        out=h[:], in0=r[:], scalar1=pb0_sb[:, kc:kc + 1],
                        scalar2=0.0, op0=mybir.AluOpType.add,
                        op1=mybir.AluOpType.max)
                    h0.append(h)
                h1p = []
                for mc in range(2):
                    p1 = pt.tile([128, BATCH], dt.float32, tag=f"p1_{mc}",
                                 name=f"p1_{mc}")
                    for kc in range(4):
                        nc.tensor.matmul(out=p1[:],
                                         lhsT=pw1_sb[kc][:, mc * 128:(mc + 1) * 128],
                                         rhs=h0[kc][:], start=(kc == 0), stop=(kc == 3))
                    h = ts.tile([128, BATCH], dt.bfloat16, tag=f"h1p_{mc}",
                                name=f"h1p_{mc}")
                    eng = nc.vector  # gpsimd cannot read PSUM
                    eng.tensor_scalar(
                        out=h[:], in0=p1[:], scalar1=pb1_sb[:, mc:mc + 1],
                        scalar2=0.0, op0=mybir.AluOpType.add,
                        op1=mybir.AluOpType.max)
                    h1p.append(h)
                p2 = pt.tile([1, BATCH], dt.float32, tag="p2", name="p2")
                for mc in range(2):
                    nc.tensor.matmul(out=p2[:], lhsT=pw2_sb[mc][:, :], rhs=h1p[mc][:],
                                     start=(mc == 0), stop=(mc == 1))
                res = ts.tile([1, BATCH], dt.float32, tag="res", name="res")
                nc.scalar.activation(out=res[:], in_=p2[:],
                                     func=mybir.ActivationFunctionType.Sigmoid,
                                     bias=pb2_sb[:, 0:1])
                nc.sync.dma_start(out=out_d[:, :], in_=res[:])

    nc.compile()
    return nc


def _host_prep(inputs):
    f32 = np.float32
    df = np.asarray(inputs["dense_features"], f32)
    sf = np.asarray(inputs["sparse_features"])
    emb = np.asarray(inputs["emb"], f32)
    pw0 = np.asarray(inputs["pw0"], f32)

    idx = ((sf.astype(np.int64) + 1) % CARD).astype(np.int32)   # [512, 26]
    embb = emb.astype(BF16)                                     # [26, CARD, 16]

    pw0v = pw0.reshape(ZR, ZR, O)
    Wfull = pw0v + pw0v.transpose(1, 0, 2)
    ar = np.arange(ZR)
    Wfull[ar, ar] = pw0v[ar, ar]
    Wb = Wfull.astype(BF16)                                     # [432, 432, 512]
    del Wfull

    dfT = np.zeros((16, BATCH), BF16)
    dfT[:13] = df.T.astype(BF16)
    dw0p = np.zeros((16, 512), f32)
    dw0p[:13] = np.asarray(inputs["dw0"], f32)

    def col(b, p):
        return np.asarray(b, f32).reshape(p, 128).T.copy()

    common = {
        "dfT": dfT,
        "dw0": dw0p.astype(BF16),
        "dw1": np.asarray(inputs["dw1"], f32).astype(BF16),
        "dw2": np.asarray(inputs["dw2"], f32).astype(BF16),
        "db0q": col(inputs["db0"], 4),
        "db1q": col(inputs["db1"], 2),
        "db2q": np.asarray(inputs["db2"], f32).reshape(64, 1).copy(),
        "pw1": np.asarray(inputs["pw1"], f32).astype(BF16),
        "pw2": np.asarray(inputs["pw2"], f32).reshape(256, 1).astype(BF16),
        "pb0q": col(inputs["pb0"], 4),
        "pb1q": col(inputs["pb1"], 2),
        "pb2q": np.asarray(inputs["pb2"], f32).reshape(1, 1).copy(),
    }
    dw3 = np.asarray(inputs["dw3"], f32).astype(BF16)
    db3 = np.asarray(inputs["db3"], f32).reshape(16, 1).astype(f32)
    zero_tab = np.zeros((CARD, ED), BF16)
    zero_idx = np.zeros(BATCH, np.int32)

    Jt = JMAP.reshape(-1)            # [NCH*128], -1 = gap
    ILt = ILMAP.reshape(-1)
    JSt = np.array([_jstart(il) for il in range(NIL)], np.int64)[ILt]

    in_maps = []
    for c in range(NCORES):
        m = dict(common)
        m["dw3"] = dw3 if c == 0 else np.zeros_like(dw3)
        m["db3q"] = db3 if c == 0 else np.zeros_like(db3)

        I = JSt + c
        Wc = Wb[I, np.maximum(Jt, 0)]        # [NCH*128, 512] bf16
        Wc[(Jt < 0) | (Jt < I)] = 0
        Wq = (Wc.astype(f32) * 256.0).astype(F8)
        m["wsb"] = np.ascontiguousarray(
            Wq.reshape(NPAIR, 2, 128, O).transpose(2, 1, 0, 3)
            .reshape(128, 2, NPAIR * O))

        m["zidx"] = np.array([[_i_of(il, c)] for il in range(NIL)], np.int32)

        # tables touched by rows [54c, 54c+54)
        units = sorted({r // 16 for r in range(54 * c, 54 * c + 54)})
        tabs = [u - 1 for u in units if u > 0]   # 0-based table indices
        assert len(tabs) <= 5
        pidx_ = np.full((96, 1), 1000, np.int32)
        if c == 0:
            pidx_[0:16, 0] = np.arange(16)
        iq = np.zeros((128, 20), np.int32)
        for j in range(5):
            if j < len(tabs):
                tj = tabs[j]
                m[f"es{j}"] = np.ascontiguousarray(embb[tj])
                icol = idx[:, tj]
                zr0 = 16 * (tj + 1)          # true z row of dim 0
                for d in range(16):
                    agr = zr0 + d - 54 * c
                    if 0 <= agr < NIL:
                        pidx_[16 + 16 * j + d, 0] = agr
            else:
                m[f"es{j}"] = zero_tab
                icol = zero_idx
            iq[:, j * 4:(j + 1) * 4] = icol.reshape(4, 128).T
        m["idxq"] = iq
        m["pidx"] = pidx_
        in_maps.append(m)
    return in_maps


def kernel(**inputs):
    from concourse import bass_utils
    import os

    if "nc" not in _state:
        _state["nc"] = _build_module()
    in_maps = _host_prep(inputs)
    trace = bool(int(os.environ.get("DLRM_TRACE", "0")))
    res = bass_utils.run_bass_kernel_spmd(
        _state["nc"], in_maps, core_ids=list(range(NCORES)), trace=trace)
    _state["last_results"] = res
    return np.asarray(res.results[0]["out"], np.float32).reshape(BATCH)
